# revision 39
# baseline (speedup 1.0000x reference)
"""Longformer attention Trainium2 kernel (8 NeuronCores, sequence-sharded).

Reference computation (B=1, L=4096, D=1024, H=16, HD=64, WINDOW=15):
  Q/K/V = x @ W{q,k,v}.T ; RoPE on Q,K ; mask = (causal & |i-j|<=7) | (j==0) | (i==0)
  out = softmax(QK^T/8 + mask) @ V @ Wo.T

Sharding: core c owns queries [c*512, (c+1)*512).  Each core receives:
  - x^T columns for {pos 0} + [start-8, start+512)  (global key + key window)
  - full transposed weights (replicated)
  - expanded RoPE tables, static band masks, V(pos 0) row (host-computed, 1MFLOP)
Each core computes its 512 output rows end-to-end (QKV proj, banded softmax
attention with global key 0, output proj).  The global query row 0 (attends to
all 4096 keys) is computed as per-core online-softmax partials over each
core's 512 owned keys and combined on the host (+ tiny 1x1024x1024 Wo gemm).
"""

import os
import numpy as np
import ml_dtypes
from contextlib import ExitStack
KDBG = os.environ.get("KDBG", "0") == "1"   # init psum for CoreSim debugging

import concourse.bass as bass
import concourse.tile as tile
from concourse import bacc, mybir
from concourse import bass_utils

P = 128
L = 4096
D = 1024
H = 16
HD = 64
NC = 8
LLOC = L // NC          # 512 queries per core
W = 521                 # x^T columns: [pos0 | 520-key window]
NT = D // P             # 8 channel tiles
NQB = LLOC // P         # 4 query blocks per core
WP = 528                # rot/cos/sin tiles padded for DVE 2x mode
BF = mybir.dt.bfloat16
F32 = mybir.dt.float32
NPBF = ml_dtypes.bfloat16

_CACHE = {}


def _build_module(loop_reps=0):
    key = ("nc", loop_reps)
    if key in _CACHE:
        return _CACHE[key]
    nc = bacc.Bacc("TRN2", target_bir_lowering=False, debug=False,
                   enable_asserts=False, num_devices=NC)

    def din(name, shape, dt=BF):
        return nc.dram_tensor(name, shape, dt, kind="ExternalInput").ap()

    xT = din("xT", [D, W])                  # [ch, W] bf16
    wqT = din("wqT", [D, D])
    wkT = din("wkT", [D, D])
    wvT = din("wvT", [D, D])
    woT = din("woT", [D, D])
    cs = din("cs", [P, W])                  # cos expanded, 2-head pattern
    sn = din("sn", [P, W])                  # sign-folded sin
    maskA = din("maskA", [P, P])            # band mask, block-diag
    maskA0 = din("maskA0", [P, P])          # first-block variant (per-core data)
    maskB = din("maskB", [P, 2 * P])        # band mask, off-diag block
    maskT = din("maskT", [8, P])            # 8-key tail corner
    v0aug = din("v0aug", [65, H * (HD + 1)])  # V row for pos 0 + ones (replicated)
    ident = din("ident", [P, P])            # identity for PE transpose

    yT = nc.dram_tensor("yT", [NT, P, LLOC], F32, kind="ExternalOutput").ap()
    o0p = nc.dram_tensor("o0p", [2, 512], F32, kind="ExternalOutput").ap()
    l0o = nc.dram_tensor("l0o", [H, 1], F32, kind="ExternalOutput").ap()
    m0o = nc.dram_tensor("m0o", [H, 1], F32, kind="ExternalOutput").ap()

    with tile.TileContext(nc) as tc, ExitStack() as ctx:
        if loop_reps:
            ctx.enter_context(tc.For_i(0, loop_reps, 1))
        const = ctx.enter_context(tc.tile_pool(name="const", bufs=1))
        big = ctx.enter_context(tc.tile_pool(name="big", bufs=1))
        work = ctx.enter_context(tc.tile_pool(name="work", bufs=3))
        att = ctx.enter_context(tc.tile_pool(name="att", bufs=4))
        row0 = ctx.enter_context(tc.tile_pool(name="row0", bufs=2))
        ps = ctx.enter_context(tc.tile_pool(name="ps", bufs=2, space="PSUM"))
        ps1 = ctx.enter_context(tc.tile_pool(name="ps1", bufs=3, space="PSUM"))
        ps2 = ctx.enter_context(tc.tile_pool(name="ps2", bufs=3, space="PSUM"))

        # ---- resident loads ----
        def load(name, ap_dram, shape, dt=BF, pat=None, pool=const, tag=None):
            t = pool.tile(shape, dt, tag=tag or name)
            nc.sync.dma_start(t[:], ap_dram if pat is None else ap_dram.rearrange(pat, p=P))
            return t

        def load_split(name, ap_dram, ncol, tag=None, pool=None):
            t = (pool or const).tile([P, NT, ncol], BF, tag=tag or name)
            r = ap_dram.rearrange("(t p) d -> p t d", p=P)
            for k in range(NT):
                nc.sync.dma_start(t[:, k, :], r[:, k, :])
            return t

        wpool = ctx.enter_context(tc.tile_pool(name="wpool", bufs=2))
        xTs = load_split("xTs", xT, W)
        wq_s = load_split("wq", wqT, D, tag="w", pool=wpool)
        wv_s = load("wv", wvT, [P, NT, D], pat="(t p) d -> p t d", pool=wpool, tag="w")
        wk_s = load("wk", wkT, [P, NT, D], pat="(t p) d -> p t d", pool=wpool, tag="w")
        wo_s = load("wo", woT, [P, NT, D], pat="(t p) d -> p t d", pool=wpool, tag="w")
        cs_s = const.tile([P, WP], BF, tag="cs")
        nc.sync.dma_start(cs_s[:, 0:W], cs[:])
        nc.vector.memset(cs_s[:, W:WP], 0.0)
        sn_s = const.tile([P, WP], BF, tag="sn")
        nc.sync.dma_start(sn_s[:, 0:W], sn[:])
        nc.vector.memset(sn_s[:, W:WP], 0.0)
        mA_s = load("mA", maskA, [P, P])
        mA0_s = load("mA0", maskA0, [P, P])
        mB_s = load("mB", maskB, [P, 2 * P])
        mT_s = load("mT", maskT, [8, P])
        v0_s = load("v0", v0aug, [65, H * (HD + 1)])
        id_s = load("ident", ident, [P, P])

        # ---- Q^T / K^T projections + RoPE ----
        # rot^T = (W @ x^T) * cos + (Perm @ (W @ x^T)) * sin_signed
        qrot = big.tile([P, NT, WP], BF)
        krot = big.tile([P, NT, WP], BF)

        H1 = 261                # balanced split of the 521 columns
        H2 = W - H1             # 260
        qtp = ctx.enter_context(tc.tile_pool(name="qtp", bufs=2))

        def project_rot(w_s, rot, after_tile=None):
            # process tiles in halves: matmuls+copies for 4 tiles, one batched
            # partition-block swap, then the rope muls (and attention callback)
            for half in range(2):
                ts_ = range(half * 4, half * 4 + 4)
                qtbig = qtp.tile([P, 4, WP], BF, tag="qtbig")
                swqb = qtp.tile([P, 4, WP], BF, tag="swqb")
                nc.vector.memset(qtbig[:, :, W:WP], 0.0)
                for i, t in enumerate(ts_):
                    pj = ps.tile([P, 512], F32, tag="sA")
                    pj2 = ps1.tile([P, 512], F32, tag="sB")
                    for k in range(NT):
                        nc.tensor.matmul(pj[:, 0:H1], lhsT=w_s[:, k, t * P:(t + 1) * P],
                                         rhs=xTs[:, k, 0:H1],
                                         start=(k == 0), stop=(k == NT - 1))
                    for k in range(NT):
                        nc.tensor.matmul(pj2[:, 0:H2], lhsT=w_s[:, k, t * P:(t + 1) * P],
                                         rhs=xTs[:, k, H1:W],
                                         start=(k == 0), stop=(k == NT - 1))
                    nc.scalar.copy(qtbig[:, i, 0:H1], pj[:, 0:H1])
                    nc.scalar.copy(qtbig[:, i, H1:W], pj2[:, 0:H2])
                for hb2 in (0, HD):
                    nc.sync.dma_start(swqb[hb2:hb2 + 32], qtbig[hb2 + 32:hb2 + HD])
                    nc.sync.dma_start(swqb[hb2 + 32:hb2 + HD], qtbig[hb2:hb2 + 32])
                for i, t in enumerate(ts_):
                    t2 = work.tile([P, WP], BF, tag="t2")
                    nc.vector.tensor_mul(t2[:], swqb[:, i, :], sn_s[:])
                    nc.vector.tensor_mul(rot[:, t, :], qtbig[:, i, :], cs_s[:])
                    nc.vector.tensor_add(rot[:, t, :], rot[:, t, :], t2[:])
                    if after_tile is not None:
                        after_tile(t)

        project_rot(wq_s, qrot)

        # ---- V projection (natural layout, keys on partitions) ----
        NVB = 5                      # 4 full key blocks + 8-key tail block
        v_s = big.tile([P, NVB, H, HD + 1], BF)
        nc.vector.memset(v_s[:, NVB - 1, :, :], 0.0)
        for j in range(NVB):
            npart = P if j < NVB - 1 else 8
            for cc in range(2):
                pv = ps.tile([P, 512], F32, tag="sA")
                for k in range(NT):
                    nc.tensor.matmul(
                        pv[0:npart, :],
                        lhsT=xTs[:, k, 1 + j * P: 1 + j * P + npart],
                        rhs=wv_s[:, k, cc * 512:(cc + 1) * 512],
                        start=(k == 0), stop=(k == NT - 1))
                nc.scalar.copy(
                    v_s[0:npart, j, cc * 8:(cc + 1) * 8, 1:1 + HD],
                    pv[0:npart, :].rearrange("p (h d) -> p h d", h=8))
        nc.vector.memset(v_s[:, :, :, 0:1], 1.0)

        # ---- banded attention, one head-unit per K-projection tile ----
        ot_all = big.tile([P, NT, LLOC], BF)
        s0 = row0.tile([H, 512], F32, tag="s0")
        EXP = mybir.ActivationFunctionType.Exp
        # q0 for all heads, zero-padded so two heads pack into one matmul
        q0all = row0.tile([P, H], BF, tag="q0a")
        nc.vector.memset(q0all[:], 0.0)
        for hp8 in range(NT):
            nc.vector.tensor_copy(q0all[0:HD, 2 * hp8:2 * hp8 + 1],
                                  qrot[0:HD, hp8, 0:1])
            nc.vector.tensor_copy(q0all[HD:P, 2 * hp8 + 1:2 * hp8 + 2],
                                  qrot[HD:P, hp8, 0:1])
        stage0 = att.tile([HD + 1, H, 256], BF, tag="stage0", name="stage0")
        stage1 = att.tile([HD + 1, H, 256], BF, tag="stage1", name="stage1")
        stages = [stage0, stage1]

        def attention_head(h):
            hb, hp = (h % 2) * HD, h // 2
            Krow = krot[hb:hb + HD, hp]
            Qrow = qrot[hb:hb + HD, hp]
            if h % 2 == 0:
                # global query row 0: scores over this core's 512 owned keys,
                # two heads per matmul via the zero-padded q0 pack
                sp = ps.tile([2, 512], F32, tag="sA")
                nc.tensor.matmul(sp[:], lhsT=q0all[:, h:h + 2],
                                 rhs=krot[:, hp, 9:W], start=True, stop=True)
                sc = row0.tile([2, 512], F32, tag="s0c")
                nc.scalar.copy(sc[:], sp[:])
                nc.sync.dma_start(s0[h:h + 2, :], sc[:])
            for c2 in range(NQB // 2):
                m = 2 * c2
                qe = slice(9 + m * P, 9 + (m + 1) * P)
                qo = slice(9 + (m + 1) * P, 9 + (m + 2) * P)
                qc = slice(9 + m * P, 9 + (m + 2) * P)
                kA = slice(1 + m * P, 1 + (m + 1) * P)
                kB = slice(1 + (m + 1) * P, 1 + (m + 2) * P)
                kT = slice(1 + (m + 2) * P, 1 + (m + 2) * P + 8)
                # one psum bank for all banded scores:
                # [B(0:256) | A(256:384) | tail rows 0:8 (384:512)]
                sS = ps1.tile([P, 512], F32, tag="sB")
                oa_t = ps2.tile([P, 512], F32, tag="oA")
                if h < 2 and c2 == 0:
                    # first use of each rotating score bank: clear the region
                    # whose unwritten rows feed exp (raw psum could be Inf ->
                    # exp -> NaN after masking); they stay finite afterwards
                    nc.vector.memset(sS[:, 384:512], 0.0)
                nc.tensor.matmul(sS[:, 0:256], lhsT=Krow[:, kB], rhs=Qrow[:, qc],
                                 start=True, stop=True)
                nc.tensor.matmul(sS[:, 256:384], lhsT=Krow[:, kA], rhs=Qrow[:, qe],
                                 start=True, stop=True)
                nc.tensor.matmul(sS[0:8, 384:512], lhsT=Krow[:, kT], rhs=Qrow[:, qo],
                                 start=True, stop=True)
                nc.tensor.matmul(oa_t[0:1, 256:512], lhsT=Krow[:, 0:1],
                                 rhs=Qrow[:, qc], start=True, stop=True)
                # exp + multiplicative band masks
                eBA = att.tile([P, 512], BF, tag="eB")
                nc.scalar.activation(eBA[:], sS[:], EXP, scale=0.125)
                eB = eBA[:, 0:256]
                eA = eBA[:, 256:512]
                eG = att.tile([1, 256], BF, tag="eG")
                nc.scalar.activation(eG[:], oa_t[0:1, 256:512], EXP, scale=0.125)
                nc.vector.tensor_mul(eB[:], eB[:], mB_s[:])
                nc.vector.tensor_mul(eA[:, 0:P], eA[:, 0:P],
                                     mA0_s[:] if m == 0 else mA_s[:])
                nc.vector.tensor_mul(eA[0:8, P:256], eA[0:8, P:256], mT_s[:])
                # O^T_aug = [1 | V]^T @ P^T   (row 0 = denominator)
                hsl = slice(h * (HD + 1), (h + 1) * (HD + 1))
                oa = oa_t[0:HD + 1, 0:256]
                nc.tensor.matmul(oa[:], lhsT=v_s[:, m + 1, h, :], rhs=eB[:],
                                 start=True, stop=False)
                nc.tensor.matmul(oa[:, 0:P], lhsT=v_s[:, m, h, :], rhs=eA[:, 0:P],
                                 start=False, stop=False)
                nc.tensor.matmul(oa[:, P:256], lhsT=v_s[0:8, m + 2, h, :],
                                 rhs=eA[0:8, P:256], start=False, stop=False)
                nc.tensor.matmul(oa[:], lhsT=v0_s[0:1, hsl],
                                 rhs=eG[:], start=False, stop=True)
                # normalize by denominator (row 0) into the chunk stage
                rsc = att.tile([1, 256], F32, tag="rsc")
                nc.vector.reciprocal_approx_fast(rsc[:], oa[0:1, :])
                rb = att.tile([HD + 1, 256], F32, tag="rb")
                nc.gpsimd.partition_broadcast(rb[:], rsc[:])
                nc.vector.tensor_mul(stages[c2][:, h, :], oa[:], rb[:])

        project_rot(wk_s, krot,
                    after_tile=lambda t: (attention_head(2 * t),
                                          attention_head(2 * t + 1)))
        for c2 in range(2):
            qsl = slice(c2 * 256, (c2 + 1) * 256)
            nc.sync.dma_start(ot_all[0:HD, :, qsl], stages[c2][1:1 + HD, 0:H:2, :])
            nc.sync.dma_start(ot_all[HD:P, :, qsl], stages[c2][1:1 + HD, 1:H:2, :])

        # ---- row-0 softmax partials + weighted-V partials ----
        m0 = row0.tile([H, 1], F32, tag="m0")
        nc.vector.tensor_reduce(m0[:], s0[:], axis=mybir.AxisListType.X,
                                op=mybir.AluOpType.max)
        m0n = row0.tile([H, 1], F32, tag="m0n")
        nc.vector.tensor_scalar_mul(m0n[:], m0[:], -0.125)
        l0 = row0.tile([H, 1], F32, tag="l0")
        e0 = row0.tile([H, NVB * P], BF, tag="e0")
        nc.vector.memset(e0[:, 0:8], 0.0)
        nc.vector.memset(e0[:, 520:NVB * P], 0.0)
        nc.scalar.activation(e0[:, 8:520], s0[:], EXP,
                             scale=0.125, bias=m0n[:], accum_out=l0[:])
        e0t = row0.tile([P, NVB, H], BF, tag="e0t")
        for j in range(NVB):
            tp = ps1.tile([P, H], BF, tag="sB")
            nc.tensor.transpose(tp[:], e0[:, j * P:(j + 1) * P], id_s[0:H, 0:H])
            nc.scalar.copy(e0t[:, j, :], tp[:])
        o0ps = ps.tile([33, 512], F32, tag="sA")
        o0a = o0ps[0:1, :]
        o0b = o0ps[32:33, :]
        for j in range(NVB):
            m2 = row0.tile([P, H, HD], BF, tag="m2")
            nc.vector.tensor_mul(m2[:], v_s[:, j, :, 1:1 + HD],
                                 e0t[:, j, :, None].to_broadcast((P, H, HD)))
            nc.tensor.matmul(o0a[:], lhsT=v_s[:, 0, 0, 0:1],
                             rhs=m2[:, 0:8, :], start=(j == 0), stop=(j == NVB - 1))
            nc.tensor.matmul(o0b[:], lhsT=v_s[:, 0, 0, 0:1],
                             rhs=m2[:, 8:H, :], start=(j == 0), stop=(j == NVB - 1))
        o0sa = row0.tile([1, 512], F32, tag="o0sa")
        o0sb = row0.tile([1, 512], F32, tag="o0sb")
        nc.scalar.copy(o0sa[:], o0a[:])
        nc.scalar.copy(o0sb[:], o0b[:])
        nc.sync.dma_start(o0p[0:1, :], o0sa[:])
        nc.sync.dma_start(o0p[1:2, :], o0sb[:])
        nc.sync.dma_start(l0o[:], l0[:])
        nc.sync.dma_start(m0o[:], m0[:])

        # ---- output projection: y^T = Wo @ O^T ----
        for t in range(NT):
            fp = ps.tile([P, 512], F32, tag="sA")
            for k in range(NT):
                nc.tensor.matmul(fp[:], lhsT=wo_s[:, k, t * P:(t + 1) * P],
                                 rhs=ot_all[:, k, :],
                                 start=(k == 0), stop=(k == NT - 1))
            fs_ = work.tile([P, 512], F32, tag="yt")
            nc.scalar.copy(fs_[:], fp[:])
            nc.sync.dma_start(yT[t], fs_[:])

    nc.compile()
    _CACHE[key] = nc
    return nc


def _host_inputs(x, Wq, Wk, Wv, Wo, freqs_cos, freqs_sin):
    x2 = np.asarray(x, np.float32).reshape(L, D)
    fc = np.asarray(freqs_cos, np.float32).reshape(L, HD // 2)
    fs = np.asarray(freqs_sin, np.float32).reshape(L, HD // 2)
    bf = lambda a: np.ascontiguousarray(a).astype(NPBF)
    # permute Q/K output channels: within each head, channel (d) -> slot
    # (d%2)*32 + d//2, so RoPE pairs occupy partition blocks [0:32|32:64]
    hperm = np.empty(D, np.int64)
    for h_ in range(H):
        for d_ in range(HD):
            hperm[h_ * HD + (d_ % 2) * 32 + d_ // 2] = h_ * HD + d_
    shared = {
        "wqT": bf(np.asarray(Wq, np.float32).T[:, hperm]),
        "wkT": bf(np.asarray(Wk, np.float32).T[:, hperm]),
        "wvT": bf(np.asarray(Wv, np.float32).T),
        "woT": bf(np.asarray(Wo, np.float32).T),
    }
    # masks: allowed iff 0 <= dq - dk <= 7 (relative positions, block-local)
    yv, xv = np.arange(P)[:, None], np.arange(P)[None, :]
    bandA = ((xv - (yv - 8) >= 0) & (xv - (yv - 8) <= 7))
    shared["maskA"] = bf(bandA.astype(np.float32))
    mA0 = bandA & (yv > 8)
    xb = np.arange(2 * P)[None, :]
    shared["maskB"] = bf(((xb - (120 + yv) >= 0) & (xb - (120 + yv) <= 7))
                         .astype(np.float32))
    yt = np.arange(8)[:, None]
    shared["maskT"] = bf(((xv - yt - 120 >= 0) & (xv - yt - 120 <= 7))
                         .astype(np.float32))
    shared["ident"] = bf(np.eye(P, dtype=np.float32))
    v0 = x2[0] @ np.asarray(Wv, np.float32).T                  # [1024]
    v0a = np.concatenate([np.ones((H, 1), np.float32),
                          v0.reshape(H, HD)], axis=1)
    shared["v0aug"] = bf(np.tile(v0a.reshape(1, H * (HD + 1)), (65, 1)))

    xT = x2.T  # [D, L]
    in_maps = []
    for c in range(NC):
        start = c * LLOC
        pos = np.arange(start - 8, start + LLOC)               # window positions
        valid = pos >= 0
        xe = np.zeros((D, W), np.float32)
        xe[:, 0] = xT[:, 0]
        xe[:, 1:][:, valid] = xT[:, pos[valid]]
        posw = np.concatenate([[0], np.clip(pos, 0, L - 1)])   # per-column position
        f = np.arange(HD) % 32                                 # permuted row -> freq
        cse = fc[posw][:, f].T                                 # [64, W]
        sne = fs[posw][:, f].T
        sgn = np.where(np.arange(HD) < 32, -1.0, 1.0)[:, None]
        cs128 = np.concatenate([cse, cse], axis=0)
        sn128 = np.concatenate([sne * sgn, sne * sgn], axis=0)
        im = dict(shared)
        im["xT"] = bf(xe)
        im["cs"] = bf(cs128)
        im["sn"] = bf(sn128)
        im["maskA0"] = bf((mA0 if c == 0 else bandA).astype(np.float32))
        in_maps.append(im)
    return in_maps


def _assemble(results, Wo):
    y = np.empty((L, D), np.float32)
    for c in range(NC):
        yt = results[c]["yT"].reshape(D, LLOC)                 # [1024, 512]
        y[c * LLOC:(c + 1) * LLOC] = yt.T
    # combine row-0 online-softmax partials
    m0 = np.stack([results[c]["m0o"].reshape(H) for c in range(NC)])   # [NC, H]
    l0 = np.stack([results[c]["l0o"].reshape(H) for c in range(NC)])
    o0 = np.stack([results[c]["o0p"].reshape(2, 8, HD).reshape(H, HD)
                   for c in range(NC)])                                # [NC, H, 64]
    mstar = m0.max(axis=0)
    alpha = np.exp(0.125 * (m0 - mstar[None]))                         # [NC, H]
    num = (alpha[:, :, None] * o0).sum(axis=0)                         # [H, 64]
    den = (alpha * l0).sum(axis=0)                                     # [H]
    row0 = (num / den[:, None]).reshape(D)
    y[0] = row0 @ np.asarray(Wo, np.float32).T
    return y.reshape(1, L, D)


def kernel(x, Wq, Wk, Wv, Wo, freqs_cos, freqs_sin):
    nc = _build_module()
    in_maps = _host_inputs(x, Wq, Wk, Wv, Wo, freqs_cos, freqs_sin)
    res = bass_utils.run_bass_kernel_spmd(nc, in_maps, core_ids=list(range(NC)))
    return _assemble(res.results, Wo)


# revision 40
# speedup vs baseline: 1.0902x; 1.0902x over previous
"""Longformer attention Trainium2 kernel (8 NeuronCores, sequence-sharded).

Reference computation (B=1, L=4096, D=1024, H=16, HD=64, WINDOW=15):
  Q/K/V = x @ W{q,k,v}.T ; RoPE on Q,K ; mask = (causal & |i-j|<=7) | (j==0) | (i==0)
  out = softmax(QK^T/8 + mask) @ V @ Wo.T

Sharding: core c owns queries [c*512, (c+1)*512).  Each core receives:
  - x^T columns for {pos 0} + [start-8, start+512)  (global key + key window)
  - full transposed weights (replicated)
  - expanded RoPE tables, static band masks, V(pos 0) row (host-computed, 1MFLOP)
Each core computes its 512 output rows end-to-end (QKV proj, banded softmax
attention with global key 0, output proj).  The global query row 0 (attends to
all 4096 keys) is computed as per-core online-softmax partials over each
core's 512 owned keys and combined on the host (+ tiny 1x1024x1024 Wo gemm).
"""

import os
import numpy as np
import ml_dtypes
from contextlib import ExitStack
KDBG = os.environ.get("KDBG", "0") == "1"   # init psum for CoreSim debugging

import concourse.bass as bass
import concourse.tile as tile
from concourse import bacc, mybir
from concourse import bass_utils

P = 128
L = 4096
D = 1024
H = 16
HD = 64
NC = 8
LLOC = L // NC          # 512 queries per core
W = 521                 # x^T columns: [pos0 | 520-key window]
NT = D // P             # 8 channel tiles
NQB = LLOC // P         # 4 query blocks per core
WP = 528                # rot/cos/sin tiles padded for DVE 2x mode
BF = mybir.dt.bfloat16
F32 = mybir.dt.float32
NPBF = ml_dtypes.bfloat16

_CACHE = {}


def _build_module(loop_reps=0):
    key = ("nc", loop_reps)
    if key in _CACHE:
        return _CACHE[key]
    nc = bacc.Bacc("TRN2", target_bir_lowering=False, debug=False,
                   enable_asserts=False, num_devices=NC)

    def din(name, shape, dt=BF):
        return nc.dram_tensor(name, shape, dt, kind="ExternalInput").ap()

    xT = din("xT", [D, W])                  # [ch, W] bf16
    wqT = din("wqT", [D, D])
    wkT = din("wkT", [D, D])
    wvT = din("wvT", [D, D])
    woT = din("woT", [D, D])
    cs = din("cs", [P, W])                  # cos expanded, 2-head pattern
    sn = din("sn", [P, W])                  # sign-folded sin
    maskA = din("maskA", [P, P])            # band mask, block-diag
    maskA0 = din("maskA0", [P, P])          # first-block variant (per-core data)
    maskB = din("maskB", [P, 2 * P])        # band mask, off-diag block
    maskT = din("maskT", [8, P])            # 8-key tail corner
    v0aug = din("v0aug", [65, H * (HD + 1)])  # V row for pos 0 + ones (replicated)
    ident = din("ident", [P, P])            # identity for PE transpose

    yT = nc.dram_tensor("yT", [NT, P, LLOC], F32, kind="ExternalOutput").ap()
    o0p = nc.dram_tensor("o0p", [2, 512], F32, kind="ExternalOutput").ap()
    l0o = nc.dram_tensor("l0o", [H, 1], F32, kind="ExternalOutput").ap()
    m0o = nc.dram_tensor("m0o", [H, 1], F32, kind="ExternalOutput").ap()

    with tile.TileContext(nc) as tc, ExitStack() as ctx:
        if loop_reps:
            ctx.enter_context(tc.For_i(0, loop_reps, 1))
        const = ctx.enter_context(tc.tile_pool(name="const", bufs=1))
        big = ctx.enter_context(tc.tile_pool(name="big", bufs=1))
        work = ctx.enter_context(tc.tile_pool(name="work", bufs=3))
        att = ctx.enter_context(tc.tile_pool(name="att", bufs=3))
        row0 = ctx.enter_context(tc.tile_pool(name="row0", bufs=2))
        ps = ctx.enter_context(tc.tile_pool(name="ps", bufs=2, space="PSUM"))
        ps1 = ctx.enter_context(tc.tile_pool(name="ps1", bufs=3, space="PSUM"))
        ps2 = ctx.enter_context(tc.tile_pool(name="ps2", bufs=3, space="PSUM"))

        # ---- resident loads ----
        def load(name, ap_dram, shape, dt=BF, pat=None, pool=const, tag=None):
            t = pool.tile(shape, dt, tag=tag or name)
            nc.sync.dma_start(t[:], ap_dram if pat is None else ap_dram.rearrange(pat, p=P))
            return t

        def load_split(name, ap_dram, ncol, tag=None, pool=None):
            t = (pool or const).tile([P, NT, ncol], BF, tag=tag or name)
            r = ap_dram.rearrange("(t p) d -> p t d", p=P)
            for k in range(NT):
                nc.sync.dma_start(t[:, k, :], r[:, k, :])
            return t

        wpool = ctx.enter_context(tc.tile_pool(name="wpool", bufs=3))
        xTs = load_split("xTs", xT, W)
        wq_s = load_split("wq", wqT, D, tag="w", pool=wpool)
        wv_s = load("wv", wvT, [P, NT, D], pat="(t p) d -> p t d", pool=wpool, tag="w")
        wk_s = load("wk", wkT, [P, NT, D], pat="(t p) d -> p t d", pool=wpool, tag="w")
        wo_s = load("wo", woT, [P, NT, D], pat="(t p) d -> p t d", pool=wpool, tag="w")
        cs_s = const.tile([P, WP], BF, tag="cs")
        nc.sync.dma_start(cs_s[:, 0:W], cs[:])
        nc.vector.memset(cs_s[:, W:WP], 0.0)
        sn_s = const.tile([P, WP], BF, tag="sn")
        nc.sync.dma_start(sn_s[:, 0:W], sn[:])
        nc.vector.memset(sn_s[:, W:WP], 0.0)
        mA_s = load("mA", maskA, [P, P])
        mA0_s = load("mA0", maskA0, [P, P])
        mB_s = load("mB", maskB, [P, 2 * P])
        mT_s = load("mT", maskT, [8, P])
        v0_s = load("v0", v0aug, [65, H * (HD + 1)])
        id_s = load("ident", ident, [P, P])

        # ---- Q^T / K^T projections + RoPE ----
        # rot^T = (W @ x^T) * cos + (Perm @ (W @ x^T)) * sin_signed
        qrot = big.tile([P, NT, WP], BF)
        krot = big.tile([P, NT, WP], BF)

        H1 = 261                # balanced split of the 521 columns
        H2 = W - H1             # 260
        qtp = ctx.enter_context(tc.tile_pool(name="qtp", bufs=2))

        def project_rot(w_s, rot, after_tile=None):
            # process tiles in halves: matmuls+copies for 4 tiles, one batched
            # partition-block swap, then the rope muls (and attention callback)
            for half in range(2):
                ts_ = range(half * 4, half * 4 + 4)
                qtbig = qtp.tile([P, 4, WP], BF, tag="qtbig")
                swqb = qtp.tile([P, 4, WP], BF, tag="swqb")
                nc.vector.memset(qtbig[:, :, W:WP], 0.0)
                for i, t in enumerate(ts_):
                    pj = ps.tile([P, 512], F32, tag="sA")
                    pj2 = ps1.tile([P, 512], F32, tag="sB")
                    for k in range(NT):
                        nc.tensor.matmul(pj[:, 0:H1], lhsT=w_s[:, k, t * P:(t + 1) * P],
                                         rhs=xTs[:, k, 0:H1],
                                         start=(k == 0), stop=(k == NT - 1))
                    for k in range(NT):
                        nc.tensor.matmul(pj2[:, 0:H2], lhsT=w_s[:, k, t * P:(t + 1) * P],
                                         rhs=xTs[:, k, H1:W],
                                         start=(k == 0), stop=(k == NT - 1))
                    nc.scalar.copy(qtbig[:, i, 0:H1], pj[:, 0:H1])
                    nc.scalar.copy(qtbig[:, i, H1:W], pj2[:, 0:H2])
                for hb2 in (0, HD):
                    nc.sync.dma_start(swqb[hb2:hb2 + 32], qtbig[hb2 + 32:hb2 + HD])
                    nc.sync.dma_start(swqb[hb2 + 32:hb2 + HD], qtbig[hb2:hb2 + 32])
                for i, t in enumerate(ts_):
                    t2 = work.tile([P, WP], BF, tag="t2")
                    nc.vector.tensor_mul(t2[:], swqb[:, i, :], sn_s[:])
                    nc.vector.tensor_mul(rot[:, t, :], qtbig[:, i, :], cs_s[:])
                    nc.vector.tensor_add(rot[:, t, :], rot[:, t, :], t2[:])
                    if after_tile is not None:
                        after_tile(t)

        project_rot(wq_s, qrot)

        # ---- V projection (natural layout, keys on partitions) ----
        NVB = 5                      # 4 full key blocks + 8-key tail block
        v_s = big.tile([P, NVB, H, HD + 1], BF)
        nc.vector.memset(v_s[:, NVB - 1, :, :], 0.0)
        for j in range(NVB):
            npart = P if j < NVB - 1 else 8
            for cc in range(2):
                pv = ps.tile([P, 512], F32, tag="sA")
                for k in range(NT):
                    nc.tensor.matmul(
                        pv[0:npart, :],
                        lhsT=xTs[:, k, 1 + j * P: 1 + j * P + npart],
                        rhs=wv_s[:, k, cc * 512:(cc + 1) * 512],
                        start=(k == 0), stop=(k == NT - 1))
                nc.scalar.copy(
                    v_s[0:npart, j, cc * 8:(cc + 1) * 8, 1:1 + HD],
                    pv[0:npart, :].rearrange("p (h d) -> p h d", h=8))
        nc.vector.memset(v_s[:, :, :, 0:1], 1.0)

        # ---- banded attention, one head-unit per K-projection tile ----
        ot_all = big.tile([P, NT, LLOC], BF)
        s0 = row0.tile([H, 512], F32, tag="s0")
        EXP = mybir.ActivationFunctionType.Exp
        # q0 for all heads, zero-padded so two heads pack into one matmul
        q0all = row0.tile([P, H], BF, tag="q0a")
        nc.vector.memset(q0all[:], 0.0)
        for hp8 in range(NT):
            nc.vector.tensor_copy(q0all[0:HD, 2 * hp8:2 * hp8 + 1],
                                  qrot[0:HD, hp8, 0:1])
            nc.vector.tensor_copy(q0all[HD:P, 2 * hp8 + 1:2 * hp8 + 2],
                                  qrot[HD:P, hp8, 0:1])
        stage0 = att.tile([HD + 1, H, 256], BF, tag="stage0", name="stage0")
        stage1 = att.tile([HD + 1, H, 256], BF, tag="stage1", name="stage1")
        stages = [stage0, stage1]

        def attention_head(h):
            hb, hp = (h % 2) * HD, h // 2
            Krow = krot[hb:hb + HD, hp]
            Qrow = qrot[hb:hb + HD, hp]
            if h % 2 == 0:
                # global query row 0: scores over this core's 512 owned keys,
                # two heads per matmul via the zero-padded q0 pack
                sp = ps.tile([2, 512], F32, tag="sA")
                nc.tensor.matmul(sp[:], lhsT=q0all[:, h:h + 2],
                                 rhs=krot[:, hp, 9:W], start=True, stop=True)
                sc = row0.tile([2, 512], F32, tag="s0c")
                nc.scalar.copy(sc[:], sp[:])
                nc.sync.dma_start(s0[h:h + 2, :], sc[:])
            for c2 in range(NQB // 2):
                m = 2 * c2
                qe = slice(9 + m * P, 9 + (m + 1) * P)
                qo = slice(9 + (m + 1) * P, 9 + (m + 2) * P)
                qc = slice(9 + m * P, 9 + (m + 2) * P)
                kA = slice(1 + m * P, 1 + (m + 1) * P)
                kB = slice(1 + (m + 1) * P, 1 + (m + 2) * P)
                kT = slice(1 + (m + 2) * P, 1 + (m + 2) * P + 8)
                # one psum bank for all banded scores:
                # [B(0:256) | A(256:384) | tail rows 0:8 (384:512)]
                sS = ps1.tile([P, 512], F32, tag="sB")
                oa_t = ps2.tile([P, 512], F32, tag="oA")
                if h < 2 and c2 == 0:
                    # first use of each rotating score bank: clear the region
                    # whose unwritten rows feed exp (raw psum could be Inf ->
                    # exp -> NaN after masking); they stay finite afterwards
                    nc.vector.memset(sS[:, 384:512], 0.0)
                nc.tensor.matmul(sS[:, 0:256], lhsT=Krow[:, kB], rhs=Qrow[:, qc],
                                 start=True, stop=True)
                nc.tensor.matmul(sS[:, 256:384], lhsT=Krow[:, kA], rhs=Qrow[:, qe],
                                 start=True, stop=True)
                nc.tensor.matmul(sS[0:8, 384:512], lhsT=Krow[:, kT], rhs=Qrow[:, qo],
                                 start=True, stop=True)
                nc.tensor.matmul(oa_t[0:1, 256:512], lhsT=Krow[:, 0:1],
                                 rhs=Qrow[:, qc], start=True, stop=True)
                # exp + multiplicative band masks
                eBA = att.tile([P, 512], BF, tag="eB")
                nc.scalar.activation(eBA[:], sS[:], EXP, scale=0.125)
                eB = eBA[:, 0:256]
                eA = eBA[:, 256:512]
                eG = att.tile([1, 256], BF, tag="eG")
                nc.scalar.activation(eG[:], oa_t[0:1, 256:512], EXP, scale=0.125)
                nc.vector.tensor_mul(eB[:], eB[:], mB_s[:])
                nc.vector.tensor_mul(eA[:, 0:P], eA[:, 0:P],
                                     mA0_s[:] if m == 0 else mA_s[:])
                nc.vector.tensor_mul(eA[0:8, P:256], eA[0:8, P:256], mT_s[:])
                # O^T_aug = [1 | V]^T @ P^T   (row 0 = denominator)
                hsl = slice(h * (HD + 1), (h + 1) * (HD + 1))
                oa = oa_t[0:HD + 1, 0:256]
                nc.tensor.matmul(oa[:], lhsT=v_s[:, m + 1, h, :], rhs=eB[:],
                                 start=True, stop=False)
                nc.tensor.matmul(oa[:, 0:P], lhsT=v_s[:, m, h, :], rhs=eA[:, 0:P],
                                 start=False, stop=False)
                nc.tensor.matmul(oa[:, P:256], lhsT=v_s[0:8, m + 2, h, :],
                                 rhs=eA[0:8, P:256], start=False, stop=False)
                nc.tensor.matmul(oa[:], lhsT=v0_s[0:1, hsl],
                                 rhs=eG[:], start=False, stop=True)
                # normalize by denominator (row 0) into the chunk stage
                rsc = att.tile([1, 256], F32, tag="rsc")
                nc.vector.reciprocal_approx_fast(rsc[:], oa[0:1, :])
                rb = att.tile([HD + 1, 256], F32, tag="rb")
                nc.gpsimd.partition_broadcast(rb[:], rsc[:])
                nc.vector.tensor_mul(stages[c2][:, h, :], oa[:], rb[:])

        project_rot(wk_s, krot,
                    after_tile=lambda t: (attention_head(2 * t),
                                          attention_head(2 * t + 1)))
        for c2 in range(2):
            qsl = slice(c2 * 256, (c2 + 1) * 256)
            nc.sync.dma_start(ot_all[0:HD, :, qsl], stages[c2][1:1 + HD, 0:H:2, :])
            nc.sync.dma_start(ot_all[HD:P, :, qsl], stages[c2][1:1 + HD, 1:H:2, :])

        # ---- row-0 softmax partials + weighted-V partials ----
        m0 = row0.tile([H, 1], F32, tag="m0")
        nc.vector.tensor_reduce(m0[:], s0[:], axis=mybir.AxisListType.X,
                                op=mybir.AluOpType.max)
        m0n = row0.tile([H, 1], F32, tag="m0n")
        nc.vector.tensor_scalar_mul(m0n[:], m0[:], -0.125)
        l0 = row0.tile([H, 1], F32, tag="l0")
        e0 = row0.tile([H, NVB * P], BF, tag="e0")
        nc.vector.memset(e0[:, 0:8], 0.0)
        nc.vector.memset(e0[:, 520:NVB * P], 0.0)
        nc.scalar.activation(e0[:, 8:520], s0[:], EXP,
                             scale=0.125, bias=m0n[:], accum_out=l0[:])
        e0t = row0.tile([P, NVB, H], BF, tag="e0t")
        for j in range(NVB):
            tp = ps1.tile([P, H], BF, tag="sB")
            nc.tensor.transpose(tp[:], e0[:, j * P:(j + 1) * P], id_s[0:H, 0:H])
            nc.scalar.copy(e0t[:, j, :], tp[:])
        o0ps = ps.tile([33, 512], F32, tag="sA")
        o0a = o0ps[0:1, :]
        o0b = o0ps[32:33, :]
        for j in range(NVB):
            m2 = row0.tile([P, H, HD], BF, tag="m2")
            nc.vector.tensor_mul(m2[:], v_s[:, j, :, 1:1 + HD],
                                 e0t[:, j, :, None].to_broadcast((P, H, HD)))
            nc.tensor.matmul(o0a[:], lhsT=v_s[:, 0, 0, 0:1],
                             rhs=m2[:, 0:8, :], start=(j == 0), stop=(j == NVB - 1))
            nc.tensor.matmul(o0b[:], lhsT=v_s[:, 0, 0, 0:1],
                             rhs=m2[:, 8:H, :], start=(j == 0), stop=(j == NVB - 1))
        o0sa = row0.tile([1, 512], F32, tag="o0sa")
        o0sb = row0.tile([1, 512], F32, tag="o0sb")
        nc.scalar.copy(o0sa[:], o0a[:])
        nc.scalar.copy(o0sb[:], o0b[:])
        nc.sync.dma_start(o0p[0:1, :], o0sa[:])
        nc.sync.dma_start(o0p[1:2, :], o0sb[:])
        nc.sync.dma_start(l0o[:], l0[:])
        nc.sync.dma_start(m0o[:], m0[:])

        # ---- output projection: y^T = Wo @ O^T ----
        for t in range(NT):
            fp = ps.tile([P, 512], F32, tag="sA")
            for k in range(NT):
                nc.tensor.matmul(fp[:], lhsT=wo_s[:, k, t * P:(t + 1) * P],
                                 rhs=ot_all[:, k, :],
                                 start=(k == 0), stop=(k == NT - 1))
            fs_ = work.tile([P, 512], F32, tag="yt")
            nc.scalar.copy(fs_[:], fp[:])
            nc.sync.dma_start(yT[t], fs_[:])

    nc.compile()
    _CACHE[key] = nc
    return nc


def _host_inputs(x, Wq, Wk, Wv, Wo, freqs_cos, freqs_sin):
    x2 = np.asarray(x, np.float32).reshape(L, D)
    fc = np.asarray(freqs_cos, np.float32).reshape(L, HD // 2)
    fs = np.asarray(freqs_sin, np.float32).reshape(L, HD // 2)
    bf = lambda a: np.ascontiguousarray(a).astype(NPBF)
    # permute Q/K output channels: within each head, channel (d) -> slot
    # (d%2)*32 + d//2, so RoPE pairs occupy partition blocks [0:32|32:64]
    hperm = np.empty(D, np.int64)
    for h_ in range(H):
        for d_ in range(HD):
            hperm[h_ * HD + (d_ % 2) * 32 + d_ // 2] = h_ * HD + d_
    shared = {
        "wqT": bf(np.asarray(Wq, np.float32).T[:, hperm]),
        "wkT": bf(np.asarray(Wk, np.float32).T[:, hperm]),
        "wvT": bf(np.asarray(Wv, np.float32).T),
        "woT": bf(np.asarray(Wo, np.float32).T),
    }
    # masks: allowed iff 0 <= dq - dk <= 7 (relative positions, block-local)
    yv, xv = np.arange(P)[:, None], np.arange(P)[None, :]
    bandA = ((xv - (yv - 8) >= 0) & (xv - (yv - 8) <= 7))
    shared["maskA"] = bf(bandA.astype(np.float32))
    mA0 = bandA & (yv > 8)
    xb = np.arange(2 * P)[None, :]
    shared["maskB"] = bf(((xb - (120 + yv) >= 0) & (xb - (120 + yv) <= 7))
                         .astype(np.float32))
    yt = np.arange(8)[:, None]
    shared["maskT"] = bf(((xv - yt - 120 >= 0) & (xv - yt - 120 <= 7))
                         .astype(np.float32))
    shared["ident"] = bf(np.eye(P, dtype=np.float32))
    v0 = x2[0] @ np.asarray(Wv, np.float32).T                  # [1024]
    v0a = np.concatenate([np.ones((H, 1), np.float32),
                          v0.reshape(H, HD)], axis=1)
    shared["v0aug"] = bf(np.tile(v0a.reshape(1, H * (HD + 1)), (65, 1)))

    xT = x2.T  # [D, L]
    in_maps = []
    for c in range(NC):
        start = c * LLOC
        pos = np.arange(start - 8, start + LLOC)               # window positions
        valid = pos >= 0
        xe = np.zeros((D, W), np.float32)
        xe[:, 0] = xT[:, 0]
        xe[:, 1:][:, valid] = xT[:, pos[valid]]
        posw = np.concatenate([[0], np.clip(pos, 0, L - 1)])   # per-column position
        f = np.arange(HD) % 32                                 # permuted row -> freq
        cse = fc[posw][:, f].T                                 # [64, W]
        sne = fs[posw][:, f].T
        sgn = np.where(np.arange(HD) < 32, -1.0, 1.0)[:, None]
        cs128 = np.concatenate([cse, cse], axis=0)
        sn128 = np.concatenate([sne * sgn, sne * sgn], axis=0)
        im = dict(shared)
        im["xT"] = bf(xe)
        im["cs"] = bf(cs128)
        im["sn"] = bf(sn128)
        im["maskA0"] = bf((mA0 if c == 0 else bandA).astype(np.float32))
        in_maps.append(im)
    return in_maps


def _assemble(results, Wo):
    y = np.empty((L, D), np.float32)
    for c in range(NC):
        yt = results[c]["yT"].reshape(D, LLOC)                 # [1024, 512]
        y[c * LLOC:(c + 1) * LLOC] = yt.T
    # combine row-0 online-softmax partials
    m0 = np.stack([results[c]["m0o"].reshape(H) for c in range(NC)])   # [NC, H]
    l0 = np.stack([results[c]["l0o"].reshape(H) for c in range(NC)])
    o0 = np.stack([results[c]["o0p"].reshape(2, 8, HD).reshape(H, HD)
                   for c in range(NC)])                                # [NC, H, 64]
    mstar = m0.max(axis=0)
    alpha = np.exp(0.125 * (m0 - mstar[None]))                         # [NC, H]
    num = (alpha[:, :, None] * o0).sum(axis=0)                         # [H, 64]
    den = (alpha * l0).sum(axis=0)                                     # [H]
    row0 = (num / den[:, None]).reshape(D)
    y[0] = row0 @ np.asarray(Wo, np.float32).T
    return y.reshape(1, L, D)


def kernel(x, Wq, Wk, Wv, Wo, freqs_cos, freqs_sin):
    nc = _build_module()
    in_maps = _host_inputs(x, Wq, Wk, Wv, Wo, freqs_cos, freqs_sin)
    res = bass_utils.run_bass_kernel_spmd(nc, in_maps, core_ids=list(range(NC)))
    return _assemble(res.results, Wo)


# revision 42
# speedup vs baseline: 1.1185x; 1.0259x over previous
"""Longformer attention Trainium2 kernel (8 NeuronCores, sequence-sharded).

Reference computation (B=1, L=4096, D=1024, H=16, HD=64, WINDOW=15):
  Q/K/V = x @ W{q,k,v}.T ; RoPE on Q,K ; mask = (causal & |i-j|<=7) | (j==0) | (i==0)
  out = softmax(QK^T/8 + mask) @ V @ Wo.T

Sharding: core c owns queries [c*512, (c+1)*512).  Each core receives:
  - x^T columns for {pos 0} + [start-8, start+512)  (global key + key window)
  - full transposed weights (replicated)
  - expanded RoPE tables, static band masks, V(pos 0) row (host-computed, 1MFLOP)
Each core computes its 512 output rows end-to-end (QKV proj, banded softmax
attention with global key 0, output proj).  The global query row 0 (attends to
all 4096 keys) is computed as per-core online-softmax partials over each
core's 512 owned keys and combined on the host (+ tiny 1x1024x1024 Wo gemm).
"""

import os
import numpy as np
import ml_dtypes
from contextlib import ExitStack
KDBG = os.environ.get("KDBG", "0") == "1"   # init psum for CoreSim debugging

import concourse.bass as bass
import concourse.tile as tile
from concourse import bacc, mybir
from concourse import bass_utils

P = 128
L = 4096
D = 1024
H = 16
HD = 64
NC = 8
LLOC = L // NC          # 512 queries per core
W = 521                 # x^T columns: [pos0 | 520-key window]
NT = D // P             # 8 channel tiles
NQB = LLOC // P         # 4 query blocks per core
WP = 528                # rot/cos/sin tiles padded for DVE 2x mode
BF = mybir.dt.bfloat16
F32 = mybir.dt.float32
NPBF = ml_dtypes.bfloat16

_CACHE = {}


def _build_module(loop_reps=0):
    key = ("nc", loop_reps)
    if key in _CACHE:
        return _CACHE[key]
    nc = bacc.Bacc("TRN2", target_bir_lowering=False, debug=False,
                   enable_asserts=False, num_devices=NC)

    def din(name, shape, dt=BF):
        return nc.dram_tensor(name, shape, dt, kind="ExternalInput").ap()

    xT = din("xT", [D, W])                  # [ch, W] bf16
    wqT = din("wqT", [D, D])
    wkT = din("wkT", [D, D])
    wvT = din("wvT", [D, D])
    woT = din("woT", [D, D])
    cs = din("cs", [P, W])                  # cos expanded, 2-head pattern
    sn = din("sn", [P, W])                  # sign-folded sin
    maskA = din("maskA", [P, P])            # band mask, block-diag
    maskA0 = din("maskA0", [P, P])          # first-block variant (per-core data)
    maskB = din("maskB", [P, 2 * P])        # band mask, off-diag block
    maskT = din("maskT", [8, P])            # 8-key tail corner
    v0aug = din("v0aug", [65, H * (HD + 1)])  # V row for pos 0 + ones (replicated)
    ident = din("ident", [P, P])            # identity for PE transpose

    yT = nc.dram_tensor("yT", [NT, P, LLOC], F32, kind="ExternalOutput").ap()
    o0p = nc.dram_tensor("o0p", [2, 512], F32, kind="ExternalOutput").ap()
    l0o = nc.dram_tensor("l0o", [H, 1], F32, kind="ExternalOutput").ap()
    m0o = nc.dram_tensor("m0o", [H, 1], F32, kind="ExternalOutput").ap()

    with tile.TileContext(nc) as tc, ExitStack() as ctx:
        if loop_reps:
            ctx.enter_context(tc.For_i(0, loop_reps, 1))
        const = ctx.enter_context(tc.tile_pool(name="const", bufs=1))
        big = ctx.enter_context(tc.tile_pool(name="big", bufs=1))
        work = ctx.enter_context(tc.tile_pool(name="work", bufs=3))
        att = ctx.enter_context(tc.tile_pool(name="att", bufs=3))
        row0 = ctx.enter_context(tc.tile_pool(name="row0", bufs=2))
        ps = ctx.enter_context(tc.tile_pool(name="ps", bufs=2, space="PSUM"))
        ps1 = ctx.enter_context(tc.tile_pool(name="ps1", bufs=3, space="PSUM"))
        ps2 = ctx.enter_context(tc.tile_pool(name="ps2", bufs=3, space="PSUM"))

        # ---- resident loads ----
        def load(name, ap_dram, shape, dt=BF, pat=None, pool=const, tag=None):
            t = pool.tile(shape, dt, tag=tag or name)
            nc.sync.dma_start(t[:], ap_dram if pat is None else ap_dram.rearrange(pat, p=P))
            return t

        def load_split(name, ap_dram, ncol, tag=None, pool=None):
            t = (pool or const).tile([P, NT, ncol], BF, tag=tag or name)
            r = ap_dram.rearrange("(t p) d -> p t d", p=P)
            for k in range(NT):
                nc.sync.dma_start(t[:, k, :], r[:, k, :])
            return t

        wpool = ctx.enter_context(tc.tile_pool(name="wpool", bufs=3))
        xTs = load_split("xTs", xT, W)
        wq_s = load_split("wq", wqT, D, tag="w", pool=wpool)
        wv_s = load("wv", wvT, [P, NT, D], pat="(t p) d -> p t d", pool=wpool, tag="w")
        wk_s = load("wk", wkT, [P, NT, D], pat="(t p) d -> p t d", pool=wpool, tag="w")
        wo_s = load("wo", woT, [P, NT, D], pat="(t p) d -> p t d", pool=wpool, tag="w")
        cs_s = const.tile([P, WP], BF, tag="cs")
        nc.sync.dma_start(cs_s[:, 0:W], cs[:])
        nc.vector.memset(cs_s[:, W:WP], 0.0)
        sn_s = const.tile([P, WP], BF, tag="sn")
        nc.sync.dma_start(sn_s[:, 0:W], sn[:])
        nc.vector.memset(sn_s[:, W:WP], 0.0)
        mA_s = load("mA", maskA, [P, P])
        mA0_s = load("mA0", maskA0, [P, P])
        mB_s = load("mB", maskB, [P, 2 * P])
        mT_s = load("mT", maskT, [8, P])
        v0_s = load("v0", v0aug, [65, H * (HD + 1)])
        id_s = load("ident", ident, [P, P])

        # ---- Q^T / K^T projections + RoPE ----
        # rot^T = (W @ x^T) * cos + (Perm @ (W @ x^T)) * sin_signed
        qrot = big.tile([P, NT, WP], BF)
        krot = big.tile([P, NT, WP], BF)

        H1 = 261                # balanced split of the 521 columns
        H2 = W - H1             # 260
        qtp = ctx.enter_context(tc.tile_pool(name="qtp", bufs=2))

        def project_rot(w_s, rot, after_tile=None):
            # process tiles in halves: matmuls+copies for 4 tiles, one batched
            # partition-block swap, then the rope muls (and attention callback)
            for half in range(2):
                ts_ = range(half * 4, half * 4 + 4)
                qtbig = qtp.tile([P, 4, WP], BF, tag="qtbig")
                swqb = qtp.tile([P, 4, WP], BF, tag="swqb")
                nc.vector.memset(qtbig[:, :, W:WP], 0.0)
                for i, t in enumerate(ts_):
                    pj = ps.tile([P, 512], F32, tag="sA")
                    pj2 = ps1.tile([P, 512], F32, tag="sB")
                    for k in range(NT):
                        nc.tensor.matmul(pj[:, 0:H1], lhsT=w_s[:, k, t * P:(t + 1) * P],
                                         rhs=xTs[:, k, 0:H1],
                                         start=(k == 0), stop=(k == NT - 1))
                    for k in range(NT):
                        nc.tensor.matmul(pj2[:, 0:H2], lhsT=w_s[:, k, t * P:(t + 1) * P],
                                         rhs=xTs[:, k, H1:W],
                                         start=(k == 0), stop=(k == NT - 1))
                    nc.scalar.copy(qtbig[:, i, 0:H1], pj[:, 0:H1])
                    nc.scalar.copy(qtbig[:, i, H1:W], pj2[:, 0:H2])
                for hb2 in (0, HD):
                    nc.sync.dma_start(swqb[hb2:hb2 + 32], qtbig[hb2 + 32:hb2 + HD])
                    nc.sync.dma_start(swqb[hb2 + 32:hb2 + HD], qtbig[hb2:hb2 + 32])
                for i, t in enumerate(ts_):
                    t2 = work.tile([P, WP], BF, tag="t2")
                    nc.vector.tensor_mul(t2[:], swqb[:, i, :], sn_s[:])
                    nc.vector.tensor_mul(rot[:, t, :], qtbig[:, i, :], cs_s[:])
                    nc.vector.tensor_add(rot[:, t, :], rot[:, t, :], t2[:])
                    if after_tile is not None:
                        after_tile(t)

        project_rot(wq_s, qrot)

        # ---- V projection (natural layout, keys on partitions) ----
        NVB = 5                      # 4 full key blocks + 8-key tail block
        v_s = big.tile([P, NVB, H, HD + 1], BF)
        nc.vector.memset(v_s[:, NVB - 1, :, :], 0.0)
        for j in range(NVB):
            npart = P if j < NVB - 1 else 8
            for cc in range(2):
                pv = ps.tile([P, 512], F32, tag="sA")
                for k in range(NT):
                    nc.tensor.matmul(
                        pv[0:npart, :],
                        lhsT=xTs[:, k, 1 + j * P: 1 + j * P + npart],
                        rhs=wv_s[:, k, cc * 512:(cc + 1) * 512],
                        start=(k == 0), stop=(k == NT - 1))
                nc.scalar.copy(
                    v_s[0:npart, j, cc * 8:(cc + 1) * 8, 1:1 + HD],
                    pv[0:npart, :].rearrange("p (h d) -> p h d", h=8))
        nc.vector.memset(v_s[:, :, :, 0:1], 1.0)

        # ---- banded attention, one head-unit per K-projection tile ----
        ot_all = big.tile([P, NT, LLOC], BF)
        s0 = row0.tile([H, 512], F32, tag="s0")
        EXP = mybir.ActivationFunctionType.Exp
        # q0 for all heads, zero-padded so two heads pack into one matmul
        q0all = row0.tile([P, H], BF, tag="q0a")
        nc.vector.memset(q0all[:], 0.0)
        for hp8 in range(NT):
            nc.vector.tensor_copy(q0all[0:HD, 2 * hp8:2 * hp8 + 1],
                                  qrot[0:HD, hp8, 0:1])
            nc.vector.tensor_copy(q0all[HD:P, 2 * hp8 + 1:2 * hp8 + 2],
                                  qrot[HD:P, hp8, 0:1])
        stage0 = att.tile([HD + 1, H, 256], BF, tag="stage0", name="stage0")
        stage1 = att.tile([HD + 1, H, 256], BF, tag="stage1", name="stage1")
        stages = [stage0, stage1]

        def attention_head(h):
            hb, hp = (h % 2) * HD, h // 2
            Krow = krot[hb:hb + HD, hp]
            Qrow = qrot[hb:hb + HD, hp]
            if h % 2 == 0:
                # global query row 0: scores over this core's 512 owned keys,
                # two heads per matmul via the zero-padded q0 pack
                sp = ps.tile([2, 512], F32, tag="sA")
                nc.tensor.matmul(sp[:], lhsT=q0all[:, h:h + 2],
                                 rhs=krot[:, hp, 9:W], start=True, stop=True)
                sc = row0.tile([2, 512], F32, tag="s0c")
                nc.scalar.copy(sc[:], sp[:])
                nc.sync.dma_start(s0[h:h + 2, :], sc[:])
            for c2 in range(NQB // 2):
                m = 2 * c2
                qe = slice(9 + m * P, 9 + (m + 1) * P)
                qo = slice(9 + (m + 1) * P, 9 + (m + 2) * P)
                qc = slice(9 + m * P, 9 + (m + 2) * P)
                kA = slice(1 + m * P, 1 + (m + 1) * P)
                kB = slice(1 + (m + 1) * P, 1 + (m + 2) * P)
                kT = slice(1 + (m + 2) * P, 1 + (m + 2) * P + 8)
                # one psum bank for all banded scores:
                # [B(0:256) | A(256:384) | tail rows 0:8 (384:512)]
                sS = ps1.tile([P, 512], F32, tag="sB")
                oa_t = ps2.tile([P, 512], F32, tag="oA")
                if h < 2 and c2 == 0:
                    # first use of each rotating score bank: clear the region
                    # whose unwritten rows feed exp (raw psum could be Inf ->
                    # exp -> NaN after masking); they stay finite afterwards
                    nc.vector.memset(sS[:, 384:512], 0.0)
                nc.tensor.matmul(sS[:, 0:256], lhsT=Krow[:, kB], rhs=Qrow[:, qc],
                                 start=True, stop=True)
                nc.tensor.matmul(sS[:, 256:384], lhsT=Krow[:, kA], rhs=Qrow[:, qe],
                                 start=True, stop=True)
                nc.tensor.matmul(sS[0:8, 384:512], lhsT=Krow[:, kT], rhs=Qrow[:, qo],
                                 start=True, stop=True)
                nc.tensor.matmul(oa_t[0:1, 256:512], lhsT=Krow[:, 0:1],
                                 rhs=Qrow[:, qc], start=True, stop=True)
                # exp + multiplicative band masks
                eBA = att.tile([P, 512], BF, tag="eB")
                nc.scalar.activation(eBA[:], sS[:], EXP, scale=0.125)
                eB = eBA[:, 0:256]
                eA = eBA[:, 256:512]
                eG = att.tile([1, 256], BF, tag="eG")
                nc.scalar.activation(eG[:], oa_t[0:1, 256:512], EXP, scale=0.125)
                nc.vector.tensor_mul(eB[:], eB[:], mB_s[:])
                nc.vector.tensor_mul(eA[:, 0:P], eA[:, 0:P],
                                     mA0_s[:] if m == 0 else mA_s[:])
                nc.vector.tensor_mul(eA[0:8, P:256], eA[0:8, P:256], mT_s[:])
                # O^T_aug = [1 | V]^T @ P^T   (row 0 = denominator)
                hsl = slice(h * (HD + 1), (h + 1) * (HD + 1))
                oa = oa_t[0:HD + 1, 0:256]
                nc.tensor.matmul(oa[:], lhsT=v_s[:, m + 1, h, :], rhs=eB[:],
                                 start=True, stop=False)
                nc.tensor.matmul(oa[:, 0:P], lhsT=v_s[:, m, h, :], rhs=eA[:, 0:P],
                                 start=False, stop=False)
                nc.tensor.matmul(oa[:, P:256], lhsT=v_s[0:8, m + 2, h, :],
                                 rhs=eA[0:8, P:256], start=False, stop=False)
                nc.tensor.matmul(oa[:], lhsT=v0_s[0:1, hsl],
                                 rhs=eG[:], start=False, stop=True)
                # normalize by denominator (row 0) into the chunk stage
                rsc = att.tile([1, 256], F32, tag="rsc")
                nc.vector.reciprocal_approx_fast(rsc[:], oa[0:1, :])
                rb = att.tile([HD + 1, 256], F32, tag="rb")
                nc.gpsimd.partition_broadcast(rb[:], rsc[:])
                nc.vector.tensor_mul(stages[c2][:, h, :], oa[:], rb[:])

        project_rot(wk_s, krot,
                    after_tile=lambda t: (attention_head(2 * t),
                                          attention_head(2 * t + 1)))
        for c2 in range(2):
            qsl = slice(c2 * 256, (c2 + 1) * 256)
            nc.sync.dma_start(ot_all[0:HD, :, qsl], stages[c2][1:1 + HD, 0:H:2, :])
            nc.sync.dma_start(ot_all[HD:P, :, qsl], stages[c2][1:1 + HD, 1:H:2, :])

        # ---- row-0 softmax partials + weighted-V partials ----
        m0 = row0.tile([H, 1], F32, tag="m0")
        nc.vector.tensor_reduce(m0[:], s0[:], axis=mybir.AxisListType.X,
                                op=mybir.AluOpType.max)
        m0n = row0.tile([H, 1], F32, tag="m0n")
        nc.vector.tensor_scalar_mul(m0n[:], m0[:], -0.125)
        l0 = row0.tile([H, 1], F32, tag="l0")
        e0 = row0.tile([H, NVB * P], BF, tag="e0")
        nc.vector.memset(e0[:, 0:8], 0.0)
        nc.vector.memset(e0[:, 520:NVB * P], 0.0)
        nc.scalar.activation(e0[:, 8:520], s0[:], EXP,
                             scale=0.125, bias=m0n[:], accum_out=l0[:])
        e0t = row0.tile([P, NVB, H], BF, tag="e0t")
        for j in range(NVB):
            tp = ps1.tile([P, H], BF, tag="sB")
            nc.tensor.transpose(tp[:], e0[:, j * P:(j + 1) * P], id_s[0:H, 0:H])
            nc.scalar.copy(e0t[:, j, :], tp[:])
        o0ps = ps.tile([33, 512], F32, tag="sA")
        o0a = o0ps[0:1, :]
        o0b = o0ps[32:33, :]
        for j in range(NVB):
            m2 = row0.tile([P, H, HD], BF, tag="m2")
            nc.vector.tensor_mul(m2[:], v_s[:, j, :, 1:1 + HD],
                                 e0t[:, j, :, None].to_broadcast((P, H, HD)))
            nc.tensor.matmul(o0a[:], lhsT=v_s[:, 0, 0, 0:1],
                             rhs=m2[:, 0:8, :], start=(j == 0), stop=(j == NVB - 1))
            nc.tensor.matmul(o0b[:], lhsT=v_s[:, 0, 0, 0:1],
                             rhs=m2[:, 8:H, :], start=(j == 0), stop=(j == NVB - 1))
        o0sa = row0.tile([1, 512], F32, tag="o0sa")
        o0sb = row0.tile([1, 512], F32, tag="o0sb")
        nc.scalar.copy(o0sa[:], o0a[:])
        nc.scalar.copy(o0sb[:], o0b[:])
        nc.sync.dma_start(o0p[0:1, :], o0sa[:])
        nc.sync.dma_start(o0p[1:2, :], o0sb[:])
        nc.sync.dma_start(l0o[:], l0[:])
        nc.sync.dma_start(m0o[:], m0[:])

        # ---- output projection: y^T = Wo @ O^T ----
        for t in range(NT):
            fp = ps.tile([P, 512], F32, tag="sA")
            for k in range(NT):
                nc.tensor.matmul(fp[:], lhsT=wo_s[:, k, t * P:(t + 1) * P],
                                 rhs=ot_all[:, k, :],
                                 start=(k == 0), stop=(k == NT - 1))
            fs_ = work.tile([P, 512], F32, tag="yt")
            nc.scalar.copy(fs_[:], fp[:])
            nc.sync.dma_start(yT[t], fs_[:])

    nc.compile()
    _CACHE[key] = nc
    return nc


def _host_inputs(x, Wq, Wk, Wv, Wo, freqs_cos, freqs_sin):
    x2 = np.asarray(x, np.float32).reshape(L, D)
    fc = np.asarray(freqs_cos, np.float32).reshape(L, HD // 2)
    fs = np.asarray(freqs_sin, np.float32).reshape(L, HD // 2)
    bf = lambda a: np.ascontiguousarray(a).astype(NPBF)
    # permute Q/K output channels: within each head, channel (d) -> slot
    # (d%2)*32 + d//2, so RoPE pairs occupy partition blocks [0:32|32:64]
    hperm = np.empty(D, np.int64)
    for h_ in range(H):
        for d_ in range(HD):
            hperm[h_ * HD + (d_ % 2) * 32 + d_ // 2] = h_ * HD + d_
    shared = {
        "wqT": bf(np.asarray(Wq, np.float32).T[:, hperm]),
        "wkT": bf(np.asarray(Wk, np.float32).T[:, hperm]),
        "wvT": bf(np.asarray(Wv, np.float32).T),
        "woT": bf(np.asarray(Wo, np.float32).T),
    }
    # masks: allowed iff 0 <= dq - dk <= 7 (relative positions, block-local)
    yv, xv = np.arange(P)[:, None], np.arange(P)[None, :]
    bandA = ((xv - (yv - 8) >= 0) & (xv - (yv - 8) <= 7))
    shared["maskA"] = bf(bandA.astype(np.float32))
    mA0 = bandA & (yv > 8)
    xb = np.arange(2 * P)[None, :]
    shared["maskB"] = bf(((xb - (120 + yv) >= 0) & (xb - (120 + yv) <= 7))
                         .astype(np.float32))
    yt = np.arange(8)[:, None]
    shared["maskT"] = bf(((xv - yt - 120 >= 0) & (xv - yt - 120 <= 7))
                         .astype(np.float32))
    shared["ident"] = bf(np.eye(P, dtype=np.float32))
    v0 = x2[0] @ np.asarray(Wv, np.float32).T                  # [1024]
    v0a = np.concatenate([np.ones((H, 1), np.float32),
                          v0.reshape(H, HD)], axis=1)
    shared["v0aug"] = bf(np.tile(v0a.reshape(1, H * (HD + 1)), (65, 1)))

    xT = x2.T  # [D, L]
    in_maps = []
    for c in range(NC):
        start = c * LLOC
        pos = np.arange(start - 8, start + LLOC)               # window positions
        valid = pos >= 0
        xe = np.zeros((D, W), np.float32)
        xe[:, 0] = xT[:, 0]
        xe[:, 1:][:, valid] = xT[:, pos[valid]]
        posw = np.concatenate([[0], np.clip(pos, 0, L - 1)])   # per-column position
        f = np.arange(HD) % 32                                 # permuted row -> freq
        cse = fc[posw][:, f].T                                 # [64, W]
        sne = fs[posw][:, f].T
        sgn = np.where(np.arange(HD) < 32, -1.0, 1.0)[:, None]
        cs128 = np.concatenate([cse, cse], axis=0)
        sn128 = np.concatenate([sne * sgn, sne * sgn], axis=0)
        im = dict(shared)
        im["xT"] = bf(xe)
        im["cs"] = bf(cs128)
        im["sn"] = bf(sn128)
        im["maskA0"] = bf((mA0 if c == 0 else bandA).astype(np.float32))
        in_maps.append(im)
    return in_maps


def _assemble(results, Wo):
    y = np.empty((L, D), np.float32)
    for c in range(NC):
        yt = results[c]["yT"].reshape(D, LLOC)                 # [1024, 512]
        y[c * LLOC:(c + 1) * LLOC] = yt.T
    # combine row-0 online-softmax partials
    m0 = np.stack([results[c]["m0o"].reshape(H) for c in range(NC)])   # [NC, H]
    l0 = np.stack([results[c]["l0o"].reshape(H) for c in range(NC)])
    o0 = np.stack([results[c]["o0p"].reshape(2, 8, HD).reshape(H, HD)
                   for c in range(NC)])                                # [NC, H, 64]
    mstar = m0.max(axis=0)
    alpha = np.exp(0.125 * (m0 - mstar[None]))                         # [NC, H]
    num = (alpha[:, :, None] * o0).sum(axis=0)                         # [H, 64]
    den = (alpha * l0).sum(axis=0)                                     # [H]
    row0 = (num / den[:, None]).reshape(D)
    y[0] = row0 @ np.asarray(Wo, np.float32).T
    return y.reshape(1, L, D)


def kernel(x, Wq, Wk, Wv, Wo, freqs_cos, freqs_sin):
    nc = _build_module()
    in_maps = _host_inputs(x, Wq, Wk, Wv, Wo, freqs_cos, freqs_sin)
    res = bass_utils.run_bass_kernel_spmd(nc, in_maps, core_ids=list(range(NC)))
    return _assemble(res.results, Wo)
